# revision 8
# baseline (speedup 1.0000x reference)
"""GCN layer on 8 trn2 NeuronCores (Bass/Tile).

Strategy (dst-range edge sharding, no collectives):
  - Core i owns dst nodes [i*6272, (i+1)*6272). Its edges are bucketed by
    64-node dst window, padded per-bucket to a uniform chunk count so the
    SPMD program is identical across cores.
  - Per 128-edge chunk: dma_gather pulls row-PAIRS (src>>1) of the f32
    feature table from HBM (int16 idx limit), a scaled one-hot over the
    dst window is built on DVE (split by src parity to select the pair
    half), and two PE matmuls accumulate agg^T per bucket into PSUM.
  - Degree normalization is folded per-edge: s_e = e_w * rsqrt(outdeg[src])
    * rsqrt(indeg[dst]) (degrees/rsqrt are index-derived metadata, computed
    host-side; e_w multiply happens on device).
  - Phase 2: h^T = [W_lin^T; b_lin]^T @ [agg^T; s_in] + W_self^T^T @ feat^T
    accumulated in PSUM, then PE-transposed and DMA'd out node-major.
"""
import numpy as np

import concourse.bass as bass
import concourse.mybir as mybir
import concourse.tile as tile
from concourse import bacc
from concourse.bass_utils import run_bass_kernel_spmd
from concourse.masks import make_identity

F32 = mybir.dt.float32
I16 = mybir.dt.int16

NCORES = 8
D = 64
W = 64           # dst window (one-hot width)
SPB = 24         # buckets per superphase (PSUM capacity)
CALL = 1024      # idxs per dma_gather call (HW ring limit)


def _plan(N, E):
    npc = -(-N // (NCORES * W)) * W          # nodes per core, multiple of W
    npad = npc * NCORES
    nb = npc // W                             # buckets per core
    nsp = -(-nb // SPB)
    return npc, npad, nb, nsp


def gcn_run(feature, e_w, src, dst, W_self, W_lin, b_lin, run_on_hw=True,
            sim_core=None):
    N, Din = feature.shape
    E = src.shape[0]
    npc, npad, nb, nsp = _plan(N, E)

    # ---------------- host prep (index metadata + sharding) ----------------
    src64 = src.astype(np.int64)
    dst64 = dst.astype(np.int64)
    out_deg = np.bincount(src64, minlength=npad).clip(1)
    in_deg = np.bincount(dst64, minlength=npad).clip(1)
    r_out = 1.0 / np.sqrt(out_deg.astype(np.float64))
    r_in = 1.0 / np.sqrt(in_deg.astype(np.float64))

    gb = dst64 // W                                    # global bucket
    order = np.argsort(gb * (2 * N) + src64, kind="stable")
    src_s = src64[order]
    dst_s = dst64[order]
    ew_s = e_w[order, 0].astype(np.float64)

    nbuckets_glob = npad // W
    cnt = np.bincount(gb, minlength=nbuckets_glob)
    bchunks = max(2, int(-(-cnt.max() // 128)))        # uniform chunks/bucket
    bslot = bchunks * 128

    # superphase structure (identical across cores)
    sp_nb = [min(SPB, nb - sp * SPB) for sp in range(nsp)]
    sp_slots_raw = [n * bslot for n in sp_nb]
    sp_slots = [-(-s // CALL) * CALL for s in sp_slots_raw]
    sp_chunks = [s // 128 for s in sp_slots]
    sp_base = np.cumsum([0] + sp_slots)
    SLOTS = int(sp_base[-1])
    total_chunks = SLOTS // 128
    ncalls = SLOTS // CALL
    C = total_chunks

    # slot index for every edge
    b_core = gb % nb
    sp_of_b = b_core // SPB
    bb_of_b = b_core % SPB
    csum = np.concatenate([[0], np.cumsum(cnt)])
    rank = np.arange(E) - csum[gb[order]]
    slot_in_core = sp_base[sp_of_b[order]] + bb_of_b[order] * bslot + rank
    core_of_edge = (gb // nb)[order]

    # per-core input arrays
    pair_rows = npad // 2
    table_pair = np.zeros((pair_rows, 2 * Din), dtype=np.float32)
    ev = feature[0::2]
    table_pair[:ev.shape[0], :Din] = ev
    od = feature[1::2]
    table_pair[:od.shape[0], Din:] = od

    def wrap128(flat):                                  # slot s -> [s%128, s//128]
        return np.ascontiguousarray(flat.reshape(C, 128).T)

    in_maps = []
    for i in range(NCORES):
        m = core_of_edge == i
        sl = slot_in_core[m].astype(np.int64)
        gidx_flat = np.zeros(SLOTS, dtype=np.int16)
        gidx_flat[sl] = (src_s[m] >> 1).astype(np.int16)
        par = (src_s[m] & 1).astype(np.int64)
        drel = (dst_s[m] % W).astype(np.float32)
        dE = np.full(SLOTS, -1.0, dtype=np.float32)
        dO = np.full(SLOTS, -1.0, dtype=np.float32)
        dE[sl[par == 0]] = drel[par == 0]
        dO[sl[par == 1]] = drel[par == 1]
        ewf = np.zeros(SLOTS, dtype=np.float32)
        ewf[sl] = ew_s[m].astype(np.float32)
        rsf = np.zeros(SLOTS, dtype=np.float32)
        rsf[sl] = (r_out[src_s[m]] * r_in[dst_s[m]]).astype(np.float32)

        gidx_w = np.zeros((128, ncalls * 64), dtype=np.int16)
        for k in range(ncalls):
            blk = gidx_flat[k * CALL:(k + 1) * CALL].reshape(64, 16).T
            gidx_w[:, k * 64:(k + 1) * 64] = np.tile(blk, (8, 1))

        lo, hi = i * npc, (i + 1) * npc
        featTs = np.zeros((Din, npc), dtype=np.float32)
        real = min(hi, N) - lo
        if real > 0:
            featTs[:, :real] = feature[lo:lo + real].T
        s_in_row = r_in[lo:hi].astype(np.float32)[None, :]

        in_maps.append({
            "table_pair": table_pair,
            "gidx": gidx_w,
            "dE": wrap128(dE), "dO": wrap128(dO),
            "ew": wrap128(ewf), "rs": wrap128(rsf),
            "iota": np.tile(np.arange(W, dtype=np.float32)[None, :], (128, 1)),
            "WlbT": np.concatenate([W_lin.T, b_lin[None, :]], 0).astype(np.float32),
            "WsT": np.ascontiguousarray(W_self.T).astype(np.float32),
            "featTs": featTs,
            "s_in": s_in_row,
        })

    # ---------------- device program (identical across cores) ----------------
    nc = bacc.Bacc("TRN2", target_bir_lowering=False, debug=False)
    t_table = nc.declare_dram_parameter("table_pair", [pair_rows, 2 * Din], F32, isOutput=False)
    t_gidx = nc.declare_dram_parameter("gidx", [128, ncalls * 64], I16, isOutput=False)
    t_dE = nc.declare_dram_parameter("dE", [128, C], F32, isOutput=False)
    t_dO = nc.declare_dram_parameter("dO", [128, C], F32, isOutput=False)
    t_ew = nc.declare_dram_parameter("ew", [128, C], F32, isOutput=False)
    t_rs = nc.declare_dram_parameter("rs", [128, C], F32, isOutput=False)
    t_iota = nc.declare_dram_parameter("iota", [128, W], F32, isOutput=False)
    t_WlbT = nc.declare_dram_parameter("WlbT", [Din + 1, D], F32, isOutput=False)
    t_WsT = nc.declare_dram_parameter("WsT", [Din, D], F32, isOutput=False)
    t_featTs = nc.declare_dram_parameter("featTs", [Din, npc], F32, isOutput=False)
    t_sin = nc.declare_dram_parameter("s_in", [1, npc], F32, isOutput=False)
    t_hout = nc.declare_dram_parameter("hout", [npc, D], F32, isOutput=True)

    with tile.TileContext(nc) as tc:
        with tc.tile_pool(name="meta", bufs=1) as meta, \
             tc.tile_pool(name="gp", bufs=3) as gp, \
             tc.tile_pool(name="ohp", bufs=8) as ohp, \
             tc.tile_pool(name="pE", bufs=3, space="PSUM") as pEp, \
             tc.tile_pool(name="pO", bufs=3, space="PSUM") as pOp, \
             tc.tile_pool(name="p2", bufs=1, space="PSUM") as p2p, \
             tc.tile_pool(name="pt", bufs=1, space="PSUM") as ptp, \
             tc.tile_pool(name="sb2", bufs=2) as sb2, \
             tc.tile_pool(name="ob", bufs=3) as ob:

            gidx_t = meta.tile([128, ncalls * 64], I16)
            nc.sync.dma_start(out=gidx_t[:], in_=t_gidx[:])
            dE_t = meta.tile([128, C], F32)
            nc.sync.dma_start(out=dE_t[:], in_=t_dE[:])
            dO_t = meta.tile([128, C], F32)
            nc.sync.dma_start(out=dO_t[:], in_=t_dO[:])
            ew_t = meta.tile([128, C], F32)
            nc.sync.dma_start(out=ew_t[:], in_=t_ew[:])
            rs_t = meta.tile([128, C], F32)
            nc.sync.dma_start(out=rs_t[:], in_=t_rs[:])
            iota_t = meta.tile([128, W], F32)
            nc.sync.dma_start(out=iota_t[:], in_=t_iota[:])
            WlbT_t = meta.tile([Din + 1, D], F32)
            nc.sync.dma_start(out=WlbT_t[:], in_=t_WlbT[:])
            WsT_t = meta.tile([Din, D], F32)
            nc.sync.dma_start(out=WsT_t[:], in_=t_WsT[:])
            featTs_t = meta.tile([Din, npc], F32)
            nc.sync.dma_start(out=featTs_t[:], in_=t_featTs[:])
            id_t = meta.tile([128, 128], F32)
            make_identity(nc, id_t[:])

            sc_t = meta.tile([128, C], F32)
            nc.vector.tensor_tensor(out=sc_t[:], in0=ew_t[:], in1=rs_t[:],
                                    op=mybir.AluOpType.mult)

            agg2 = meta.tile([Din + 1, npc], F32)
            nc.sync.dma_start(out=agg2[Din:Din + 1, :], in_=t_sin[:])

            # ---- main loop: one PSUM bank pair per bucket, drain at bucket end ----
            chunk0 = 0
            call0 = 0
            psE = psO = None
            for sp in range(nsp):
                nbs = sp_nb[sp]
                nch = sp_chunks[sp]
                for cc in range(nch):
                    c = chunk0 + cc
                    if cc % 8 == 0:
                        g = gp.tile([128, 8, 2 * Din], F32, tag="g")
                        k = call0 + cc // 8
                        nc.gpsimd.dma_gather(
                            out_ap=g[:], in_ap=t_table[:],
                            idxs_ap=gidx_t[:, k * 64:(k + 1) * 64],
                            num_idxs=CALL, num_idxs_reg=CALL,
                            elem_size=2 * Din, single_packet=False)
                    bb = min(cc // bchunks, nbs - 1)
                    kk = cc - bb * bchunks
                    start = kk == 0
                    stop = (cc == nch - 1) or (bb < nbs - 1 and kk == bchunks - 1)
                    if start:
                        psE = pEp.tile([D, W], F32, tag="pse")
                        psO = pOp.tile([D, W], F32, tag="pso")
                    ohE = ohp.tile([128, W], F32, tag="ohE")
                    nc.vector.tensor_scalar(
                        out=ohE[:], in0=iota_t[:],
                        scalar1=dE_t[:, c:c + 1], scalar2=sc_t[:, c:c + 1],
                        op0=mybir.AluOpType.is_equal, op1=mybir.AluOpType.mult)
                    ohO = ohp.tile([128, W], F32, tag="ohO")
                    nc.vector.tensor_scalar(
                        out=ohO[:], in0=iota_t[:],
                        scalar1=dO_t[:, c:c + 1], scalar2=sc_t[:, c:c + 1],
                        op0=mybir.AluOpType.is_equal, op1=mybir.AluOpType.mult)
                    nc.tensor.matmul(out=psE[:], lhsT=g[:, cc % 8, 0:Din],
                                     rhs=ohE[:], start=start, stop=stop)
                    nc.tensor.matmul(out=psO[:], lhsT=g[:, cc % 8, Din:2 * Din],
                                     rhs=ohO[:], start=start, stop=stop)
                    if stop:
                        b = sp * SPB + bb
                        nc.vector.tensor_copy(
                            out=agg2[0:Din, b * W:(b + 1) * W], in_=psE[:])
                        nc.vector.tensor_tensor(
                            out=agg2[0:Din, b * W:(b + 1) * W],
                            in0=agg2[0:Din, b * W:(b + 1) * W],
                            in1=psO[:], op=mybir.AluOpType.add)
                chunk0 += nch
                call0 += nch // 8

            # ---- phase 2: h^T chunks, transpose, store ----
            widths = []
            pos = 0
            while pos < npc:
                w = min(512, npc - pos)
                widths.append((pos, w))
                pos += w
            for (pos, wd) in widths:
                ps2 = p2p.tile([D, 512], F32, tag="p2")
                nc.tensor.matmul(out=ps2[:, :wd], lhsT=WlbT_t[:],
                                 rhs=agg2[:, pos:pos + wd], start=True, stop=False)
                nc.tensor.matmul(out=ps2[:, :wd], lhsT=WsT_t[:],
                                 rhs=featTs_t[:, pos:pos + wd], start=False, stop=True)
                hT = sb2.tile([D, 512], F32, tag="hT")
                nc.vector.tensor_copy(out=hT[:, :wd], in_=ps2[:, :wd])
                for g0 in range(0, wd, 128):
                    gw = min(128, wd - g0)
                    pt = ptp.tile([128, D], F32, tag="pt")
                    nc.tensor.transpose(out=pt[:gw, :], in_=hT[:, g0:g0 + gw],
                                        identity=id_t[:Din, :Din])
                    hsb = ob.tile([128, D], F32, tag="hsb")
                    nc.vector.tensor_copy(out=hsb[:gw, :], in_=pt[:gw, :])
                    nc.sync.dma_start(out=t_hout[pos + g0:pos + g0 + gw, :],
                                      in_=hsb[:gw, :])
    nc.compile()

    if sim_core is not None:
        from concourse.bass_interp import CoreSim
        sim = CoreSim(nc, trace=False)
        for k, v in in_maps[sim_core].items():
            sim.tensor(k)[:] = v
        sim.simulate(check_with_hw=False)
        return np.asarray(sim.tensor("hout")).copy(), None

    res = run_bass_kernel_spmd(nc, in_maps, list(range(NCORES)),
                               trace=run_on_hw == "trace")
    h_full = np.concatenate([np.asarray(res.results[i]["hout"]) for i in range(NCORES)], axis=0)
    return h_full[:N], res


def kernel(feature, e_w, snorm_n, snorm_e, src, dst, W_self, W_lin, b_lin):
    h, _ = gcn_run(np.asarray(feature, dtype=np.float32),
                   np.asarray(e_w, dtype=np.float32),
                   np.asarray(src), np.asarray(dst),
                   np.asarray(W_self, dtype=np.float32),
                   np.asarray(W_lin, dtype=np.float32),
                   np.asarray(b_lin, dtype=np.float32))
    return (h, np.asarray(e_w, dtype=np.float32))


# revision 11
# speedup vs baseline: 1.1485x; 1.1485x over previous
"""GCN layer on 8 trn2 NeuronCores (Bass/Tile).

Strategy (dst-range edge sharding, no collectives):
  - Core i owns dst nodes [i*6272, (i+1)*6272). Its edges are bucketed by
    64-node dst window, padded per-bucket to a uniform chunk count so the
    SPMD program is identical across cores.
  - Per 128-edge chunk: dma_gather pulls row-PAIRS (src>>1) of the f32
    feature table from HBM (int16 idx limit), a scaled one-hot over the
    dst window is built on DVE (split by src parity to select the pair
    half), and two PE matmuls accumulate agg^T per bucket into PSUM.
  - Degree normalization is folded per-edge: s_e = e_w * rsqrt(outdeg[src])
    * rsqrt(indeg[dst]) (degrees/rsqrt are index-derived metadata, computed
    host-side; e_w multiply happens on device).
  - Phase 2: h^T = [W_lin^T; b_lin]^T @ [agg^T; s_in] + W_self^T^T @ feat^T
    accumulated in PSUM, then PE-transposed and DMA'd out node-major.
"""
import numpy as np

import concourse.bass as bass
import concourse.mybir as mybir
import concourse.tile as tile
from concourse import bacc
from concourse.bass_utils import run_bass_kernel_spmd
from concourse.masks import make_identity

F32 = mybir.dt.float32
I16 = mybir.dt.int16

NCORES = 8
D = 64
W = 64           # dst window (one-hot width)
SPB = 24         # buckets per superphase (PSUM capacity)
CALL = 1024      # idxs per dma_gather call (HW ring limit)


def _plan(N, E):
    npc = -(-N // (NCORES * W)) * W          # nodes per core, multiple of W
    npad = npc * NCORES
    nb = npc // W                             # buckets per core
    nsp = -(-nb // SPB)
    return npc, npad, nb, nsp


def gcn_run(feature, e_w, src, dst, W_self, W_lin, b_lin, run_on_hw=True,
            sim_core=None):
    N, Din = feature.shape
    E = src.shape[0]
    npc, npad, nb, nsp = _plan(N, E)

    # ---------------- host prep (index metadata + sharding) ----------------
    src64 = src.astype(np.int64)
    dst64 = dst.astype(np.int64)
    out_deg = np.bincount(src64, minlength=npad).clip(1)
    in_deg = np.bincount(dst64, minlength=npad).clip(1)
    r_out = 1.0 / np.sqrt(out_deg.astype(np.float64))
    r_in = 1.0 / np.sqrt(in_deg.astype(np.float64))

    gb = dst64 // W                                    # global bucket
    order = np.argsort(gb * (2 * N) + src64, kind="stable")
    src_s = src64[order]
    dst_s = dst64[order]
    ew_s = e_w[order, 0].astype(np.float64)

    nbuckets_glob = npad // W
    cnt = np.bincount(gb, minlength=nbuckets_glob)
    bchunks = max(2, int(-(-cnt.max() // 128)))        # uniform chunks/bucket
    bslot = bchunks * 128

    # superphase structure (identical across cores)
    sp_nb = [min(SPB, nb - sp * SPB) for sp in range(nsp)]
    sp_slots_raw = [n * bslot for n in sp_nb]
    sp_slots = [-(-s // CALL) * CALL for s in sp_slots_raw]
    sp_chunks = [s // 128 for s in sp_slots]
    sp_base = np.cumsum([0] + sp_slots)
    SLOTS = int(sp_base[-1])
    total_chunks = SLOTS // 128
    ncalls = SLOTS // CALL
    C = total_chunks

    # slot index for every edge
    b_core = gb % nb
    sp_of_b = b_core // SPB
    bb_of_b = b_core % SPB
    csum = np.concatenate([[0], np.cumsum(cnt)])
    rank = np.arange(E) - csum[gb[order]]
    slot_in_core = sp_base[sp_of_b[order]] + bb_of_b[order] * bslot + rank
    core_of_edge = (gb // nb)[order]

    # per-core input arrays
    pair_rows = npad // 2
    table_pair = np.zeros((pair_rows, 2 * Din), dtype=np.float32)
    ev = feature[0::2]
    table_pair[:ev.shape[0], :Din] = ev
    od = feature[1::2]
    table_pair[:od.shape[0], Din:] = od

    def wrap128(flat):                                  # slot s -> [s%128, s//128]
        return np.ascontiguousarray(flat.reshape(C, 128).T)

    in_maps = []
    for i in range(NCORES):
        m = core_of_edge == i
        sl = slot_in_core[m].astype(np.int64)
        gidx_flat = np.zeros(SLOTS, dtype=np.int16)
        gidx_flat[sl] = (src_s[m] >> 1).astype(np.int16)
        par = (src_s[m] & 1).astype(np.int64)
        drel = (dst_s[m] % W).astype(np.float32)
        dE = np.full(SLOTS, -1.0, dtype=np.float32)
        dO = np.full(SLOTS, -1.0, dtype=np.float32)
        dE[sl[par == 0]] = drel[par == 0]
        dO[sl[par == 1]] = drel[par == 1]
        ewf = np.zeros(SLOTS, dtype=np.float32)
        ewf[sl] = ew_s[m].astype(np.float32)
        rsf = np.zeros(SLOTS, dtype=np.float32)
        rsf[sl] = (r_out[src_s[m]] * r_in[dst_s[m]]).astype(np.float32)

        gidx_w = np.zeros((128, ncalls * 64), dtype=np.int16)
        for k in range(ncalls):
            blk = gidx_flat[k * CALL:(k + 1) * CALL].reshape(64, 16).T
            gidx_w[:, k * 64:(k + 1) * 64] = np.tile(blk, (8, 1))

        lo, hi = i * npc, (i + 1) * npc
        featTs = np.zeros((Din, npc), dtype=np.float32)
        real = min(hi, N) - lo
        if real > 0:
            featTs[:, :real] = feature[lo:lo + real].T
        s_in_row = r_in[lo:hi].astype(np.float32)[None, :]

        in_maps.append({
            "table_pair": table_pair,
            "gidx": gidx_w,
            "dE": wrap128(dE), "dO": wrap128(dO),
            "ew": wrap128(ewf), "rs": wrap128(rsf),
            "iota": np.tile(np.arange(W, dtype=np.float32)[None, :], (128, 1)),
            "WlbT": np.concatenate([W_lin.T, b_lin[None, :]], 0).astype(np.float32),
            "WsT": np.ascontiguousarray(W_self.T).astype(np.float32),
            "featTs": featTs,
            "s_in": s_in_row,
        })

    # ---------------- device program (identical across cores) ----------------
    nc = bacc.Bacc("TRN2", target_bir_lowering=False, debug=False)
    t_table = nc.declare_dram_parameter("table_pair", [pair_rows, 2 * Din], F32, isOutput=False)
    t_gidx = nc.declare_dram_parameter("gidx", [128, ncalls * 64], I16, isOutput=False)
    t_dE = nc.declare_dram_parameter("dE", [128, C], F32, isOutput=False)
    t_dO = nc.declare_dram_parameter("dO", [128, C], F32, isOutput=False)
    t_ew = nc.declare_dram_parameter("ew", [128, C], F32, isOutput=False)
    t_rs = nc.declare_dram_parameter("rs", [128, C], F32, isOutput=False)
    t_iota = nc.declare_dram_parameter("iota", [128, W], F32, isOutput=False)
    t_WlbT = nc.declare_dram_parameter("WlbT", [Din + 1, D], F32, isOutput=False)
    t_WsT = nc.declare_dram_parameter("WsT", [Din, D], F32, isOutput=False)
    t_featTs = nc.declare_dram_parameter("featTs", [Din, npc], F32, isOutput=False)
    t_sin = nc.declare_dram_parameter("s_in", [1, npc], F32, isOutput=False)
    t_hout = nc.declare_dram_parameter("hout", [npc, D], F32, isOutput=True)

    with tile.TileContext(nc) as tc:
        with tc.tile_pool(name="meta", bufs=1) as meta, \
             tc.tile_pool(name="gp", bufs=3) as gp, \
             tc.tile_pool(name="ohp", bufs=3) as ohp, \
             tc.tile_pool(name="pE", bufs=3, space="PSUM") as pEp, \
             tc.tile_pool(name="pO", bufs=3, space="PSUM") as pOp, \
             tc.tile_pool(name="p2", bufs=1, space="PSUM") as p2p, \
             tc.tile_pool(name="pt", bufs=1, space="PSUM") as ptp, \
             tc.tile_pool(name="sb2", bufs=2) as sb2, \
             tc.tile_pool(name="ob", bufs=3) as ob:

            # split per-superphase so the first gather / first one-hot only
            # waits on its own slice
            sp_cbase = np.cumsum([0] + sp_chunks)
            gidx_sp, dE_sp, dO_sp, sc_sp = [], [], [], []
            for sp in range(nsp):
                ch0, ch1 = int(sp_cbase[sp]), int(sp_cbase[sp + 1])
                k0, k1 = ch0 // 8, ch1 // 8
                gt = meta.tile([128, (k1 - k0) * 64], I16, tag=f"gidx{sp}")
                nc.sync.dma_start(out=gt[:], in_=t_gidx[:, k0 * 64:k1 * 64])
                gidx_sp.append(gt)
                de = meta.tile([128, ch1 - ch0], F32, tag=f"dE{sp}")
                nc.sync.dma_start(out=de[:], in_=t_dE[:, ch0:ch1])
                dE_sp.append(de)
                do = meta.tile([128, ch1 - ch0], F32, tag=f"dO{sp}")
                nc.sync.dma_start(out=do[:], in_=t_dO[:, ch0:ch1])
                dO_sp.append(do)
                ewt = meta.tile([128, ch1 - ch0], F32, tag=f"ew{sp}")
                nc.sync.dma_start(out=ewt[:], in_=t_ew[:, ch0:ch1])
                rst = meta.tile([128, ch1 - ch0], F32, tag=f"rs{sp}")
                nc.sync.dma_start(out=rst[:], in_=t_rs[:, ch0:ch1])
                sct = meta.tile([128, ch1 - ch0], F32, tag=f"sc{sp}")
                nc.vector.tensor_tensor(out=sct[:], in0=ewt[:], in1=rst[:],
                                        op=mybir.AluOpType.mult)
                sc_sp.append(sct)
            iota_t = meta.tile([128, W], F32)
            nc.sync.dma_start(out=iota_t[:], in_=t_iota[:])
            WlbT_t = meta.tile([Din + 1, D], F32)
            nc.sync.dma_start(out=WlbT_t[:], in_=t_WlbT[:])
            WsT_t = meta.tile([Din, D], F32)
            nc.sync.dma_start(out=WsT_t[:], in_=t_WsT[:])
            featTs_t = meta.tile([Din, npc], F32)
            nc.sync.dma_start(out=featTs_t[:], in_=t_featTs[:])
            id_t = meta.tile([128, 128], F32)
            make_identity(nc, id_t[:])

            agg2 = meta.tile([Din + 1, npc], F32)
            nc.sync.dma_start(out=agg2[Din:Din + 1, :], in_=t_sin[:])

            def phase2_group(pos, wd):
                ps2 = p2p.tile([D, 512], F32, tag="p2")
                nc.tensor.matmul(out=ps2[:, :wd], lhsT=WlbT_t[:],
                                 rhs=agg2[:, pos:pos + wd], start=True, stop=False)
                nc.tensor.matmul(out=ps2[:, :wd], lhsT=WsT_t[:],
                                 rhs=featTs_t[:, pos:pos + wd], start=False, stop=True)
                hT = sb2.tile([D, 512], F32, tag="hT")
                nc.vector.tensor_copy(out=hT[:, :wd], in_=ps2[:, :wd])
                for g0 in range(0, wd, 128):
                    gw = min(128, wd - g0)
                    pt = ptp.tile([128, D], F32, tag="pt")
                    nc.tensor.transpose(out=pt[:gw, :], in_=hT[:, g0:g0 + gw],
                                        identity=id_t[:Din, :Din])
                    hsb = ob.tile([128, D], F32, tag="hsb")
                    nc.vector.tensor_copy(out=hsb[:gw, :], in_=pt[:gw, :])
                    nc.sync.dma_start(out=t_hout[pos + g0:pos + g0 + gw, :],
                                      in_=hsb[:gw, :])

            def bcast(ap, n_rep, axis):
                # insert a 0-step dim of size n_rep at `axis` of a 2D AP view
                pattern = [ap.ap[0]] + ([[0, n_rep], ap.ap[1]] if axis == 1
                                        else [ap.ap[1], [0, n_rep]])
                return bass.AP(ap.tensor, ap.offset, pattern)

            # ---- main loop: one PSUM bank pair per bucket, drain at bucket end,
            #      phase-2 interleaved per 8 completed buckets ----
            chunk0 = 0
            psE = psO = None
            nb_done = 0
            for sp in range(nsp):
                nbs = sp_nb[sp]
                nch = sp_chunks[sp]
                for cc in range(nch):
                    if cc % 8 == 0:
                        g = gp.tile([128, 8, 2 * Din], F32, tag="g")
                        k = cc // 8
                        nc.gpsimd.dma_gather(
                            out_ap=g[:], in_ap=t_table[:],
                            idxs_ap=gidx_sp[sp][:, k * 64:(k + 1) * 64],
                            num_idxs=CALL, num_idxs_reg=CALL,
                            elem_size=2 * Din, single_packet=False)
                        # batched scaled one-hots for the 8 chunks of this call
                        ohE = ohp.tile([128, 8, W], F32, tag="ohE")
                        ohO = ohp.tile([128, 8, W], F32, tag="ohO")
                        it_b = bcast(iota_t[:], 8, 1)
                        for oh, dsp in ((ohE, dE_sp[sp]), (ohO, dO_sp[sp])):
                            nc.vector.tensor_tensor(
                                out=oh[:], in0=it_b,
                                in1=bcast(dsp[:, cc:cc + 8], W, 2),
                                op=mybir.AluOpType.is_equal)
                            nc.vector.tensor_tensor(
                                out=oh[:], in0=oh[:],
                                in1=bcast(sc_sp[sp][:, cc:cc + 8], W, 2),
                                op=mybir.AluOpType.mult)
                    bb = min(cc // bchunks, nbs - 1)
                    kk = cc - bb * bchunks
                    start = kk == 0
                    stop = (cc == nch - 1) or (bb < nbs - 1 and kk == bchunks - 1)
                    if start:
                        psE = pEp.tile([D, W], F32, tag="pse")
                        psO = pOp.tile([D, W], F32, tag="pso")
                    nc.tensor.matmul(out=psE[:], lhsT=g[:, cc % 8, 0:Din],
                                     rhs=ohE[:, cc % 8, :], start=start, stop=stop)
                    nc.tensor.matmul(out=psO[:], lhsT=g[:, cc % 8, Din:2 * Din],
                                     rhs=ohO[:, cc % 8, :], start=start, stop=stop)
                    if stop:
                        b = sp * SPB + bb
                        nc.vector.tensor_copy(
                            out=agg2[0:Din, b * W:(b + 1) * W], in_=psE[:])
                        nc.vector.tensor_tensor(
                            out=agg2[0:Din, b * W:(b + 1) * W],
                            in0=agg2[0:Din, b * W:(b + 1) * W],
                            in1=psO[:], op=mybir.AluOpType.add)
                        nb_done += 1
                        if nb_done % 8 == 0:
                            phase2_group((nb_done - 8) * W, 8 * W)
                        elif nb_done == nb:
                            rem = nb % 8
                            phase2_group((nb - rem) * W, rem * W)
                chunk0 += nch
    nc.compile()

    if sim_core is not None:
        from concourse.bass_interp import CoreSim
        sim = CoreSim(nc, trace=False)
        for k, v in in_maps[sim_core].items():
            sim.tensor(k)[:] = v
        sim.simulate(check_with_hw=False)
        return np.asarray(sim.tensor("hout")).copy(), None

    res = run_bass_kernel_spmd(nc, in_maps, list(range(NCORES)),
                               trace=run_on_hw == "trace")
    h_full = np.concatenate([np.asarray(res.results[i]["hout"]) for i in range(NCORES)], axis=0)
    return h_full[:N], res


def kernel(feature, e_w, snorm_n, snorm_e, src, dst, W_self, W_lin, b_lin):
    h, _ = gcn_run(np.asarray(feature, dtype=np.float32),
                   np.asarray(e_w, dtype=np.float32),
                   np.asarray(src), np.asarray(dst),
                   np.asarray(W_self, dtype=np.float32),
                   np.asarray(W_lin, dtype=np.float32),
                   np.asarray(b_lin, dtype=np.float32))
    return (h, np.asarray(e_w, dtype=np.float32))


# revision 13
# speedup vs baseline: 1.1862x; 1.0328x over previous
"""GCN layer on 8 trn2 NeuronCores (Bass/Tile).

Strategy (dst-range edge sharding, no collectives):
  - Core i owns dst nodes [i*6272, (i+1)*6272). Its edges are bucketed by
    64-node dst window, padded per-bucket to a uniform chunk count so the
    SPMD program is identical across cores.
  - Per 128-edge chunk: dma_gather pulls row-PAIRS (src>>1) of the f32
    feature table from HBM (int16 idx limit), a scaled one-hot over the
    dst window is built on DVE (split by src parity to select the pair
    half), and two PE matmuls accumulate agg^T per bucket into PSUM.
  - Degree normalization is folded per-edge: s_e = e_w * rsqrt(outdeg[src])
    * rsqrt(indeg[dst]) (degrees/rsqrt are index-derived metadata, computed
    host-side; e_w multiply happens on device).
  - Phase 2: h^T = [W_lin^T; b_lin]^T @ [agg^T; s_in] + W_self^T^T @ feat^T
    accumulated in PSUM, then PE-transposed and DMA'd out node-major.
"""
import numpy as np

import concourse.bass as bass
import concourse.mybir as mybir
import concourse.tile as tile
from concourse import bacc
from concourse.bass_utils import run_bass_kernel_spmd
from concourse.masks import make_identity

F32 = mybir.dt.float32
I16 = mybir.dt.int16

NCORES = 8
D = 64
W = 64           # dst window (one-hot width)
SPB = 24         # buckets per superphase (PSUM capacity)
CALL = 1536      # idxs per dma_gather call (HW-validated cap)
CPC = CALL // 128  # chunks per call
IW = CALL // 16    # gidx cols per call


def _plan(N, E):
    npc = -(-N // (NCORES * W)) * W          # nodes per core, multiple of W
    npad = npc * NCORES
    nb = npc // W                             # buckets per core
    nsp = -(-nb // SPB)
    return npc, npad, nb, nsp


def gcn_run(feature, e_w, src, dst, W_self, W_lin, b_lin, run_on_hw=True,
            sim_core=None):
    N, Din = feature.shape
    E = src.shape[0]
    npc, npad, nb, nsp = _plan(N, E)

    # ---------------- host prep (index metadata + sharding) ----------------
    src64 = src.astype(np.int64)
    dst64 = dst.astype(np.int64)
    out_deg = np.bincount(src64, minlength=npad).clip(1)
    in_deg = np.bincount(dst64, minlength=npad).clip(1)
    r_out = 1.0 / np.sqrt(out_deg.astype(np.float64))
    r_in = 1.0 / np.sqrt(in_deg.astype(np.float64))

    gb = dst64 // W                                    # global bucket
    order = np.argsort(gb * (2 * N) + src64, kind="stable")
    src_s = src64[order]
    dst_s = dst64[order]
    ew_s = e_w[order, 0].astype(np.float64)

    nbuckets_glob = npad // W
    cnt = np.bincount(gb, minlength=nbuckets_glob)
    bchunks = max(2, int(-(-cnt.max() // 128)))        # uniform chunks/bucket
    bslot = bchunks * 128

    # superphase structure (identical across cores)
    sp_nb = [min(SPB, nb - sp * SPB) for sp in range(nsp)]
    sp_slots_raw = [n * bslot for n in sp_nb]
    sp_slots = [-(-s // CALL) * CALL for s in sp_slots_raw]
    sp_chunks = [s // 128 for s in sp_slots]
    sp_base = np.cumsum([0] + sp_slots)
    SLOTS = int(sp_base[-1])
    total_chunks = SLOTS // 128
    ncalls = SLOTS // CALL
    C = total_chunks

    # slot index for every edge
    b_core = gb % nb
    sp_of_b = b_core // SPB
    bb_of_b = b_core % SPB
    csum = np.concatenate([[0], np.cumsum(cnt)])
    rank = np.arange(E) - csum[gb[order]]
    slot_in_core = sp_base[sp_of_b[order]] + bb_of_b[order] * bslot + rank
    core_of_edge = (gb // nb)[order]

    # per-core input arrays
    pair_rows = npad // 2
    table_pair = np.zeros((pair_rows, 2 * Din), dtype=np.float32)
    ev = feature[0::2]
    table_pair[:ev.shape[0], :Din] = ev
    od = feature[1::2]
    table_pair[:od.shape[0], Din:] = od

    def wrap128(flat):                                  # slot s -> [s%128, s//128]
        return np.ascontiguousarray(flat.reshape(C, 128).T)

    in_maps = []
    for i in range(NCORES):
        m = core_of_edge == i
        sl = slot_in_core[m].astype(np.int64)
        gidx_flat = np.zeros(SLOTS, dtype=np.int16)
        gidx_flat[sl] = (src_s[m] >> 1).astype(np.int16)
        par = (src_s[m] & 1).astype(np.int64)
        drel = (dst_s[m] % W).astype(np.float32)
        dE = np.full(SLOTS, -1.0, dtype=np.float32)
        dO = np.full(SLOTS, -1.0, dtype=np.float32)
        dE[sl[par == 0]] = drel[par == 0]
        dO[sl[par == 1]] = drel[par == 1]
        ewf = np.zeros(SLOTS, dtype=np.float32)
        ewf[sl] = ew_s[m].astype(np.float32)
        rsf = np.zeros(SLOTS, dtype=np.float32)
        rsf[sl] = (r_out[src_s[m]] * r_in[dst_s[m]]).astype(np.float32)

        gidx_w = np.zeros((128, ncalls * IW), dtype=np.int16)
        for k in range(ncalls):
            blk = gidx_flat[k * CALL:(k + 1) * CALL].reshape(CALL // 16, 16).T
            gidx_w[:, k * IW:(k + 1) * IW] = np.tile(blk, (8, 1))

        lo, hi = i * npc, (i + 1) * npc
        featTs = np.zeros((Din, npc), dtype=np.float32)
        real = min(hi, N) - lo
        if real > 0:
            featTs[:, :real] = feature[lo:lo + real].T
        s_in_row = r_in[lo:hi].astype(np.float32)[None, :]

        in_maps.append({
            "table_pair": table_pair,
            "gidx": gidx_w,
            "dE": wrap128(dE), "dO": wrap128(dO),
            "ew": wrap128(ewf), "rs": wrap128(rsf),
            "iota": np.tile(np.arange(W, dtype=np.float32)[None, :], (128, 1)),
            "WlbT": np.concatenate([W_lin.T, b_lin[None, :]], 0).astype(np.float32),
            "WsT": np.ascontiguousarray(W_self.T).astype(np.float32),
            "featTs": featTs,
            "s_in": s_in_row,
        })

    # ---------------- device program (identical across cores) ----------------
    nc = bacc.Bacc("TRN2", target_bir_lowering=False, debug=False,
                   dynamic_dma_scratch_size=49152)
    t_table = nc.declare_dram_parameter("table_pair", [pair_rows, 2 * Din], F32, isOutput=False)
    t_gidx = nc.declare_dram_parameter("gidx", [128, ncalls * IW], I16, isOutput=False)
    t_dE = nc.declare_dram_parameter("dE", [128, C], F32, isOutput=False)
    t_dO = nc.declare_dram_parameter("dO", [128, C], F32, isOutput=False)
    t_ew = nc.declare_dram_parameter("ew", [128, C], F32, isOutput=False)
    t_rs = nc.declare_dram_parameter("rs", [128, C], F32, isOutput=False)
    t_iota = nc.declare_dram_parameter("iota", [128, W], F32, isOutput=False)
    t_WlbT = nc.declare_dram_parameter("WlbT", [Din + 1, D], F32, isOutput=False)
    t_WsT = nc.declare_dram_parameter("WsT", [Din, D], F32, isOutput=False)
    t_featTs = nc.declare_dram_parameter("featTs", [Din, npc], F32, isOutput=False)
    t_sin = nc.declare_dram_parameter("s_in", [1, npc], F32, isOutput=False)
    t_hout = nc.declare_dram_parameter("hout", [npc, D], F32, isOutput=True)

    with tile.TileContext(nc) as tc:
        with tc.tile_pool(name="meta", bufs=1) as meta, \
             tc.tile_pool(name="gp", bufs=3) as gp, \
             tc.tile_pool(name="ohp", bufs=3) as ohp, \
             tc.tile_pool(name="pE", bufs=3, space="PSUM") as pEp, \
             tc.tile_pool(name="pO", bufs=3, space="PSUM") as pOp, \
             tc.tile_pool(name="p2", bufs=1, space="PSUM") as p2p, \
             tc.tile_pool(name="pt", bufs=1, space="PSUM") as ptp, \
             tc.tile_pool(name="sb2", bufs=2) as sb2, \
             tc.tile_pool(name="ob", bufs=3) as ob:

            # split per-superphase so the first gather / first one-hot only
            # waits on its own slice
            sp_cbase = np.cumsum([0] + sp_chunks)
            gidx_sp, dE_sp, dO_sp, sc_sp = [], [], [], []
            for sp in range(nsp):
                ch0, ch1 = int(sp_cbase[sp]), int(sp_cbase[sp + 1])
                k0, k1 = ch0 // CPC, ch1 // CPC
                gt = meta.tile([128, (k1 - k0) * IW], I16, tag=f"gidx{sp}")
                nc.sync.dma_start(out=gt[:], in_=t_gidx[:, k0 * IW:k1 * IW])
                gidx_sp.append(gt)
                de = meta.tile([128, ch1 - ch0], F32, tag=f"dE{sp}")
                nc.sync.dma_start(out=de[:], in_=t_dE[:, ch0:ch1])
                dE_sp.append(de)
                do = meta.tile([128, ch1 - ch0], F32, tag=f"dO{sp}")
                nc.sync.dma_start(out=do[:], in_=t_dO[:, ch0:ch1])
                dO_sp.append(do)
                ewt = meta.tile([128, ch1 - ch0], F32, tag=f"ew{sp}")
                nc.sync.dma_start(out=ewt[:], in_=t_ew[:, ch0:ch1])
                rst = meta.tile([128, ch1 - ch0], F32, tag=f"rs{sp}")
                nc.sync.dma_start(out=rst[:], in_=t_rs[:, ch0:ch1])
                sct = meta.tile([128, ch1 - ch0], F32, tag=f"sc{sp}")
                nc.vector.tensor_tensor(out=sct[:], in0=ewt[:], in1=rst[:],
                                        op=mybir.AluOpType.mult)
                sc_sp.append(sct)
            iota_t = meta.tile([128, W], F32)
            nc.sync.dma_start(out=iota_t[:], in_=t_iota[:])
            WlbT_t = meta.tile([Din + 1, D], F32)
            nc.sync.dma_start(out=WlbT_t[:], in_=t_WlbT[:])
            WsT_t = meta.tile([Din, D], F32)
            nc.sync.dma_start(out=WsT_t[:], in_=t_WsT[:])
            featTs_t = meta.tile([Din, npc], F32)
            nc.sync.dma_start(out=featTs_t[:], in_=t_featTs[:])
            id_t = meta.tile([128, 128], F32)
            make_identity(nc, id_t[:])

            agg2 = meta.tile([Din + 1, npc], F32)
            nc.sync.dma_start(out=agg2[Din:Din + 1, :], in_=t_sin[:])

            def phase2_group(pos, wd):
                ps2 = p2p.tile([D, 512], F32, tag="p2")
                nc.tensor.matmul(out=ps2[:, :wd], lhsT=WlbT_t[:],
                                 rhs=agg2[:, pos:pos + wd], start=True, stop=False)
                nc.tensor.matmul(out=ps2[:, :wd], lhsT=WsT_t[:],
                                 rhs=featTs_t[:, pos:pos + wd], start=False, stop=True)
                hT = sb2.tile([D, 512], F32, tag="hT")
                nc.vector.tensor_copy(out=hT[:, :wd], in_=ps2[:, :wd])
                for g0 in range(0, wd, 128):
                    gw = min(128, wd - g0)
                    pt = ptp.tile([128, D], F32, tag="pt")
                    nc.tensor.transpose(out=pt[:gw, :], in_=hT[:, g0:g0 + gw],
                                        identity=id_t[:Din, :Din])
                    hsb = ob.tile([128, D], F32, tag="hsb")
                    nc.vector.tensor_copy(out=hsb[:gw, :], in_=pt[:gw, :])
                    nc.sync.dma_start(out=t_hout[pos + g0:pos + g0 + gw, :],
                                      in_=hsb[:gw, :])

            def bcast(ap, n_rep, axis):
                # insert a 0-step dim of size n_rep at `axis` of a 2D AP view
                pattern = [ap.ap[0]] + ([[0, n_rep], ap.ap[1]] if axis == 1
                                        else [ap.ap[1], [0, n_rep]])
                return bass.AP(ap.tensor, ap.offset, pattern)

            # ---- main loop: one PSUM bank pair per bucket, drain at bucket end,
            #      phase-2 interleaved per 8 completed buckets ----
            chunk0 = 0
            psE = psO = None
            nb_done = 0
            for sp in range(nsp):
                nbs = sp_nb[sp]
                nch = sp_chunks[sp]
                for cc in range(nch):
                    if cc % CPC == 0:
                        g = gp.tile([128, CPC, 2 * Din], F32, tag="g")
                        k = cc // CPC
                        nc.gpsimd.dma_gather(
                            out_ap=g[:], in_ap=t_table[:],
                            idxs_ap=gidx_sp[sp][:, k * IW:(k + 1) * IW],
                            num_idxs=CALL, num_idxs_reg=CALL,
                            elem_size=2 * Din, single_packet=False)
                        # batched scaled one-hots for this call's chunks
                        ohE = ohp.tile([128, CPC, W], F32, tag="ohE")
                        ohO = ohp.tile([128, CPC, W], F32, tag="ohO")
                        it_b = bcast(iota_t[:], CPC, 1)
                        for oh, dsp in ((ohE, dE_sp[sp]), (ohO, dO_sp[sp])):
                            nc.vector.tensor_tensor(
                                out=oh[:], in0=it_b,
                                in1=bcast(dsp[:, cc:cc + CPC], W, 2),
                                op=mybir.AluOpType.is_equal)
                            nc.vector.tensor_tensor(
                                out=oh[:], in0=oh[:],
                                in1=bcast(sc_sp[sp][:, cc:cc + CPC], W, 2),
                                op=mybir.AluOpType.mult)
                    bb = min(cc // bchunks, nbs - 1)
                    kk = cc - bb * bchunks
                    start = kk == 0
                    stop = (cc == nch - 1) or (bb < nbs - 1 and kk == bchunks - 1)
                    if start:
                        psE = pEp.tile([D, W], F32, tag="pse")
                        psO = pOp.tile([D, W], F32, tag="pso")
                    nc.tensor.matmul(out=psE[:], lhsT=g[:, cc % CPC, 0:Din],
                                     rhs=ohE[:, cc % CPC, :], start=start, stop=stop)
                    nc.tensor.matmul(out=psO[:], lhsT=g[:, cc % CPC, Din:2 * Din],
                                     rhs=ohO[:, cc % CPC, :], start=start, stop=stop)
                    if stop:
                        b = sp * SPB + bb
                        nc.vector.tensor_copy(
                            out=agg2[0:Din, b * W:(b + 1) * W], in_=psE[:])
                        nc.vector.tensor_tensor(
                            out=agg2[0:Din, b * W:(b + 1) * W],
                            in0=agg2[0:Din, b * W:(b + 1) * W],
                            in1=psO[:], op=mybir.AluOpType.add)
                        nb_done += 1
                        if nb_done % 8 == 0:
                            phase2_group((nb_done - 8) * W, 8 * W)
                        elif nb_done == nb:
                            rem = nb % 8
                            phase2_group((nb - rem) * W, rem * W)
                chunk0 += nch
    nc.compile()

    if sim_core is not None:
        from concourse.bass_interp import CoreSim
        sim = CoreSim(nc, trace=False)
        for k, v in in_maps[sim_core].items():
            sim.tensor(k)[:] = v
        sim.simulate(check_with_hw=False)
        return np.asarray(sim.tensor("hout")).copy(), None

    res = run_bass_kernel_spmd(nc, in_maps, list(range(NCORES)),
                               trace=run_on_hw == "trace")
    h_full = np.concatenate([np.asarray(res.results[i]["hout"]) for i in range(NCORES)], axis=0)
    return h_full[:N], res


def kernel(feature, e_w, snorm_n, snorm_e, src, dst, W_self, W_lin, b_lin):
    h, _ = gcn_run(np.asarray(feature, dtype=np.float32),
                   np.asarray(e_w, dtype=np.float32),
                   np.asarray(src), np.asarray(dst),
                   np.asarray(W_self, dtype=np.float32),
                   np.asarray(W_lin, dtype=np.float32),
                   np.asarray(b_lin, dtype=np.float32))
    return (h, np.asarray(e_w, dtype=np.float32))
